# revision 2
# baseline (speedup 1.0000x reference)
"""Multi-head attention (B=4, S=2048, D=1024, 16 heads x 64) on 8 trn2 cores.

Sharding: core c handles batch b = c//2 and head-group hg = c%2 (8 heads each,
i.e. columns hg*512:(hg+1)*512 of Wq/Wk/Wv and rows of Wo).  Each core returns
a partial output [S, D]; the host sums the two partials per batch and adds bo.

Per-core kernel (everything "T" = feature-on-partition layout):
  phase 1: QT = (Wq.T @ Xq.T)+bq  [512, S],  KT likewise, V = Xv@Wv+bv [S, 520]
           (V stored in 65-wide head groups: 64 value cols + a ones column).
           Inputs arrive host-transposed as xT [1024, S] so the contraction
           dim (d_model) is already on partitions; matmuls run in float32r
           (full fp32 numerics, full PE rate at free-dim >= 256).
  phase 2: per head-pair hp and 512-col query chunk c:
           scoresT[sk,q] = KT_h.T @ QT_h via row-paired (64+64) matmuls,
           exp fused into the PSUM->SBUF eviction on ScalarE (scale=1/8),
           probsT stored bf16.  zT~[65, 512] = V~_h.T @ probsT accumulated
           over the 16 key tiles; row 64 is the softmax denominator (ones
           column).  Normalize: reciprocal (DVE) -> partition-broadcast (DMA)
           -> multiply during PSUM eviction -> ZT [512, S] fp32.
  phase 3: out = Z @ Wo, natural [S, D] layout, DMA'd to DRAM.
"""

import numpy as np

import concourse.bass as bass
import concourse.tile as tile
from concourse import bacc, mybir
from concourse.bass_utils import run_bass_kernel_spmd

F32 = mybir.dt.float32
F32R = mybir.dt.float32r
BF16 = mybir.dt.bfloat16
ACT = mybir.ActivationFunctionType

D = 1024          # d_model
HH = 512          # heads-per-core * head_dim = 8 * 64
HD = 64           # head dim
NHL = 8           # heads per core
B, S_FULL = 4, 2048
N_CORES = 8


def build_nc(S=S_FULL, debug_taps=False, upto=3, reps=1):
    """Build the per-core Bass program (same program for all 8 cores).

    reps > 1 repeats the whole computation inside one program (for timing:
    per-exec HW time is the slope between two reps variants)."""
    assert reps == 1 or not debug_taps
    nc = bacc.Bacc("TRN2", target_bir_lowering=False, debug=False,
                   dynamic_dma_scratch_size=2048)

    xqT = nc.dram_tensor("xqT", [D, S], F32, kind="ExternalInput").ap()
    xkT = nc.dram_tensor("xkT", [D, S], F32, kind="ExternalInput").ap()
    xvT = nc.dram_tensor("xvT", [D, S], F32, kind="ExternalInput").ap()
    wq = nc.dram_tensor("wq", [D, HH], F32, kind="ExternalInput").ap()
    wk = nc.dram_tensor("wk", [D, HH], F32, kind="ExternalInput").ap()
    wv = nc.dram_tensor("wv", [D, HH], F32, kind="ExternalInput").ap()
    wo = nc.dram_tensor("wo", [HH, D], F32, kind="ExternalInput").ap()
    bq = nc.dram_tensor("bq", [HH], F32, kind="ExternalInput").ap()
    bk = nc.dram_tensor("bk", [HH], F32, kind="ExternalInput").ap()
    bv = nc.dram_tensor("bv", [HH], F32, kind="ExternalInput").ap()
    out = nc.dram_tensor("out", [S, D], F32, kind="ExternalOutput").ap()

    NT = S // 512        # 512-token chunks
    NSK = S // 128       # 128-token key tiles
    NKT = D // 128       # 128-wide d_model tiles
    NKB = HH // 128      # 128-wide hidden tiles (also head pairs)

    with tile.TileContext(nc) as tc:
        from contextlib import ExitStack

        for rep in range(reps):
          with ExitStack() as ctx:
            persist = ctx.enter_context(tc.tile_pool(name="persist", bufs=1))
            qt_sb = persist.tile([128, NKB, S], F32R, tag="qt")
            kt_sb = persist.tile([128, NKB, S], F32R, tag="kt")
            vb_sb = persist.tile([128, NSK, NHL * (HD + 1)], BF16, tag="vb")
            zt_sb = persist.tile([128, NKB, S], F32R, tag="zt")
            wo_sb = persist.tile([128, NKB, D], F32R, tag="wo")
            bq_sb = persist.tile([128, NKB], F32, tag="bq")
            bk_sb = persist.tile([128, NKB], F32, tag="bk")
            bvb_sb = persist.tile([128, HH], F32, tag="bvb")

            nc.sync.dma_start(out=bq_sb, in_=bq.rearrange("(kb p) -> p kb", p=128))
            nc.sync.dma_start(out=bk_sb, in_=bk.rearrange("(kb p) -> p kb", p=128))
            bv_bcast_in = bass.AP(tensor=bv.tensor, offset=bv.offset,
                                  ap=[[0, 128], [1, HH]])
            nc.sync.dma_start(out=bvb_sb, in_=bv_bcast_in)
            # ones columns of V~ (softmax denominator trick)
            ones_view = vb_sb.rearrange("p s (h dd) -> p s h dd", dd=HD + 1)[:, :, :, HD:HD + 1]
            nc.vector.memset(ones_view, 1.0)

            # ---------------- phase 1: projections ----------------
            with ExitStack() as c1:
                wpool = c1.enter_context(tc.tile_pool(name="wpool", bufs=2))
                xpool = c1.enter_context(tc.tile_pool(name="xpool", bufs=3))
                p1 = c1.enter_context(tc.tile_pool(name="p1", bufs=4, space="PSUM"))

                # K and Q passes: produce KT/QT [hidden-on-partition, tokens].
                # K first: scores for query-chunk c need full KT but only
                # chunk c of QT, so attention starts while Q still streams.
                for (xT, w_dram, dst, bias) in ((xkT, wk, kt_sb, bk_sb),
                                                (xqT, wq, qt_sb, bq_sb)):
                    w_sb = wpool.tile([128, NKT, HH], F32R, tag="w",
                                      name=f"w_{dst.name}")
                    nc.sync.dma_start(out=w_sb,
                                      in_=w_dram.bitcast(F32R).rearrange("(kt p) n -> p kt n", p=128))
                    for t in range(NT):
                        xt = xpool.tile([128, NKT, 512], F32R, tag="xt")
                        nc.sync.dma_start(
                            out=xt,
                            in_=xT.bitcast(F32R).rearrange("(kt p) s -> p kt s", p=128)[:, :, t * 512:(t + 1) * 512])
                        for kb in range(NKB):
                            ps = p1.tile([128, 512], F32, tag="ps1")
                            for kt in range(NKT):
                                nc.tensor.matmul(
                                    ps,
                                    lhsT=w_sb[:, kt, kb * 128:(kb + 1) * 128],
                                    rhs=xt[:, kt, :],
                                    start=(kt == 0), stop=(kt == NKT - 1))
                            nc.vector.tensor_scalar_add(
                                dst[:, kb, t * 512:(t + 1) * 512], ps,
                                bias[:, kb:kb + 1])

                # V pass: natural [tokens, hidden] with 65-wide head groups
                wv_sb = wpool.tile([128, NKT, HH], F32R, tag="w", name="w_v")
                nc.sync.dma_start(out=wv_sb,
                                  in_=wv.bitcast(F32R).rearrange("(kt p) n -> p kt n", p=128))
                for t in range(NT):
                    xt = xpool.tile([128, NKT, 512], F32R, tag="xt")
                    nc.sync.dma_start(
                        out=xt,
                        in_=xvT.bitcast(F32R).rearrange("(kt p) s -> p kt s", p=128)[:, :, t * 512:(t + 1) * 512])
                    for m in range(4):
                        ps = p1.tile([128, 512], F32, tag="ps1")
                        for kt in range(NKT):
                            nc.tensor.matmul(
                                ps,
                                lhsT=xt[:, kt, m * 128:(m + 1) * 128],
                                rhs=wv_sb[:, kt, :],
                                start=(kt == 0), stop=(kt == NKT - 1))
                        sk = t * 4 + m
                        vdst = vb_sb[:, sk, :].rearrange(
                            "p (h dd) -> p h dd", dd=HD + 1)[:, :, 0:HD]
                        nc.vector.tensor_add(
                            vdst,
                            ps.rearrange("p (h d) -> p h d", d=HD),
                            bvb_sb.rearrange("p (h d) -> p h d", d=HD))

            if upto == 1:
                fill = persist.tile([128, D], F32, tag="fill")
                nc.vector.memset(fill, 0.0)
                for t in range(S // 128):
                    nc.sync.dma_start(out=out[t * 128:(t + 1) * 128, :], in_=fill)
            if upto >= 2:
                # ---------------- phase 2: attention ----------------
                with ExitStack() as c2:
                    ptpool = c2.enter_context(tc.tile_pool(name="ptpool", bufs=2))
                    spool = c2.enter_context(tc.tile_pool(name="spool", bufs=1, space="PSUM"))
                    zpool = c2.enter_context(tc.tile_pool(name="zpool", bufs=1, space="PSUM"))
                    rpool = c2.enter_context(tc.tile_pool(name="rpool", bufs=3))

                    for hp in range(NKB):
                        for c in range(NT):
                            zps = [zpool.tile([HD + 1, 512], F32, tag=f"z{d}",
                                              name=f"zps{d}_{hp}_{c}")
                                   for d in range(2)]
                            pts = [ptpool.tile([128, NSK, 512], BF16, tag=f"pt{d}",
                                               name=f"pts{d}_{hp}_{c}")
                                   for d in range(2)]
                            for g0 in range(0, NSK, 3):
                                gs = min(3, NSK - g0)
                                for d in range(2):
                                    sp = spool.tile([128, 3, 512], F32, tag=f"s{d}")
                                    for j in range(gs):
                                        sk = g0 + j
                                        nc.tensor.matmul(
                                            sp[:, j, :],
                                            lhsT=kt_sb[d * 64:(d + 1) * 64, hp,
                                                       sk * 128:(sk + 1) * 128],
                                            rhs=qt_sb[d * 64:(d + 1) * 64, hp,
                                                      c * 512:(c + 1) * 512],
                                            start=True, stop=True)
                                    nc.scalar.activation(
                                        pts[d][:, g0:g0 + gs, :], sp[:, :gs, :],
                                        ACT.Exp, scale=0.125)
                                for d in range(2):
                                    h = 2 * hp + d
                                    for j in range(gs):
                                        sk = g0 + j
                                        nc.tensor.matmul(
                                            zps[d],
                                            lhsT=vb_sb[:, sk, h * (HD + 1):(h + 1) * (HD + 1)],
                                            rhs=pts[d][:, sk, :],
                                            start=(sk == 0), stop=(sk == NSK - 1))
                            for d in range(2):
                                # evict z~ to SBUF immediately so the PSUM bank
                                # frees for the next chunk; normalize from SBUF
                                zr = rpool.tile([HD + 1, 512], F32, tag="zr")
                                nc.vector.tensor_copy(zr, zps[d])
                                # custom-DVE recip can't read base_partition 64:
                                # stage the denominator row at partition 0 first
                                dn = rpool.tile([1, 512], F32, tag="dn")
                                nc.vector.tensor_copy(dn, zr[HD:HD + 1, :])
                                rc = rpool.tile([1, 512], F32, tag="rc")
                                nc.vector.reciprocal_approx_fast(rc, dn)
                                bc = rpool.tile([HD, 512], F32, tag="bc")
                                nc.gpsimd.partition_broadcast(bc, rc, channels=HD)
                                nc.vector.tensor_mul(
                                    zt_sb[d * 64:d * 64 + HD, hp, c * 512:(c + 1) * 512],
                                    zr[0:HD, :], bc)

            if debug_taps:
                NSKl = S // 128
                qt_d = nc.dram_tensor("qt_d", [128, NKB, S], F32, kind="ExternalOutput").ap()
                kt_d = nc.dram_tensor("kt_d", [128, NKB, S], F32, kind="ExternalOutput").ap()
                vb_d = nc.dram_tensor("vb_d", [128, NSKl, NHL * (HD + 1)], F32, kind="ExternalOutput").ap()
                zt_d = nc.dram_tensor("zt_d", [128, NKB, S], F32, kind="ExternalOutput").ap()
                with tc.tile_pool(name="dbg", bufs=1) as dbg:
                    vb_f = dbg.tile([128, NSKl, NHL * (HD + 1)], F32)
                    nc.vector.tensor_copy(vb_f, vb_sb)
                    nc.sync.dma_start(out=vb_d, in_=vb_f)
                nc.sync.dma_start(out=qt_d, in_=qt_sb.bitcast(F32))
                nc.sync.dma_start(out=kt_d, in_=kt_sb.bitcast(F32))
                nc.sync.dma_start(out=zt_d, in_=zt_sb.bitcast(F32))

            if upto >= 3:
                # ---------------- phase 3: output projection ----------------
                # wo loads here (not at kernel start) to keep the early DMA
                # window clear for xkT/xqT, which gate the first scores
                nc.sync.dma_start(out=wo_sb, in_=wo.bitcast(F32R).rearrange("(hb p) n -> p hb n", p=128))
                with ExitStack() as c3:
                    opool = c3.enter_context(tc.tile_pool(name="opool", bufs=3))
                    p3 = c3.enter_context(tc.tile_pool(name="p3", bufs=3, space="PSUM"))
                    for t in range(S // 128):
                        os_t = opool.tile([128, D], F32, tag="os")
                        for n in range(D // 512):
                            po = p3.tile([128, 512], F32, tag="po")
                            for hb in range(NKB):
                                nc.tensor.matmul(
                                    po,
                                    lhsT=zt_sb[:, hb, t * 128:(t + 1) * 128],
                                    rhs=wo_sb[:, hb, n * 512:(n + 1) * 512],
                                    start=(hb == 0), stop=(hb == NKB - 1))
                            nc.vector.tensor_copy(os_t[:, n * 512:(n + 1) * 512], po)
                        nc.sync.dma_start(out=out[t * 128:(t + 1) * 128, :], in_=os_t)

    nc.compile()
    return nc


_NC_CACHE = {}


def _get_nc(S=S_FULL):
    if S not in _NC_CACHE:
        _NC_CACHE[S] = build_nc(S)
    return _NC_CACHE[S]


def make_in_maps(query, key, value, Wq, bq, Wk, bk, Wv, bv, Wo, bo):
    """Shard full inputs into 8 per-core input dicts."""
    f32 = lambda a: np.ascontiguousarray(np.asarray(a, dtype=np.float32))
    in_maps = []
    for core in range(N_CORES):
        b, hg = core // 2, core % 2
        sl = slice(hg * HH, (hg + 1) * HH)
        in_maps.append({
            "xqT": f32(np.asarray(query)[b].T),
            "xkT": f32(np.asarray(key)[b].T),
            "xvT": f32(np.asarray(value)[b].T),
            "wq": f32(np.asarray(Wq)[:, sl]),
            "wk": f32(np.asarray(Wk)[:, sl]),
            "wv": f32(np.asarray(Wv)[:, sl]),
            "wo": f32(np.asarray(Wo)[sl, :]),
            "bq": f32(np.asarray(bq)[sl]),
            "bk": f32(np.asarray(bk)[sl]),
            "bv": f32(np.asarray(bv)[sl]),
        })
    return in_maps


def kernel(query, key, value, Wq, bq, Wk, bk, Wv, bv, Wo, bo, **run_kwargs):
    nc = _get_nc(S_FULL)
    in_maps = make_in_maps(query, key, value, Wq, bq, Wk, bk, Wv, bv, Wo, bo)
    res = run_bass_kernel_spmd(nc, in_maps, core_ids=list(range(N_CORES)),
                               **run_kwargs)
    bo_np = np.asarray(bo, dtype=np.float32)
    outs = [np.asarray(r["out"], dtype=np.float32) for r in res.results]
    full = np.stack([outs[2 * b] + outs[2 * b + 1] + bo_np for b in range(B)])
    return full.astype(np.float32)



# revision 12
# speedup vs baseline: 2.1505x; 2.1505x over previous
"""Multi-head attention (B=4, S=2048, D=1024, 16 heads x 64) on 8 trn2 cores.

Sharding: core c handles batch b = c//2 and head-group hg = c%2 (8 heads each,
i.e. columns hg*512:(hg+1)*512 of Wq/Wk/Wv and rows of Wo).  Each core returns
a partial output [S, D]; the host sums the two partials per batch and adds bo.

Per-core kernel, structured so the ScalarE (exp) stream starts as early as
possible and stays saturated -- the 33.6M exp()/core on ACT at 1 elem/cycle
(~219us) is the pacing engine; all PE work that is off the exp critical path
(V/Q projections, out-projection) is emitted as filler the Tile scheduler
slots into exp-wait gaps:

  emission order: KT(all) -> QT(c0) -> V(all) -> attention(c0)@high_priority
                  -> [attention(c) ; QT(c+1) ; outproj(c-1)] for c=1..3
                  -> outproj(3)

  attention(hp, c): per 2-key-tile group: 4 score matmuls emitted
  d0/d1-alternating (lhsT base partitions 0/64 -> distinct PE row groups ->
  the pair runs concurrently on HW), exp fused into PSUM->SBUF eviction on
  ScalarE (scale=1/8, bf16 out), then zT accumulation (K=128) over key
  tiles; V~ carries a ones column (65-wide head groups) so the softmax
  denominator rides along in row 64.  Normalize: reciprocal (DVE) ->
  partition-broadcast (GpSimd) -> multiply into ZT.

  PSUM budget (8 banks): score pair 2x[128,2,512]=4, zT 2x[65,512]=2,
  projections 1, out-projection 1.
"""

import numpy as np
import jax.numpy as jnp

import concourse.bass as bass
import concourse.tile as tile
from concourse import bacc, mybir
from concourse.bass_utils import run_bass_kernel_spmd

F32 = mybir.dt.float32
F32R = mybir.dt.float32r
BF16 = mybir.dt.bfloat16
ACT = mybir.ActivationFunctionType

D = 1024          # d_model
HH = 512          # heads-per-core * head_dim = 8 * 64
HD = 64           # head dim
NHL = 8           # heads per core
B, S_FULL = 4, 2048
N_CORES = 8


def build_nc(S=S_FULL, reps=1):
    """Build the per-core Bass program (same program for all 8 cores).

    reps > 1 repeats the whole computation inside one program (for timing:
    per-exec HW time is the slope between two reps variants)."""
    nc = bacc.Bacc("TRN2", target_bir_lowering=False, debug=False,
                   dynamic_dma_scratch_size=2048)

    xqT = nc.dram_tensor("xqT", [D, S], BF16, kind="ExternalInput").ap()
    xkT = nc.dram_tensor("xkT", [D, S], BF16, kind="ExternalInput").ap()
    xvT = nc.dram_tensor("xvT", [D, S], BF16, kind="ExternalInput").ap()
    wq = nc.dram_tensor("wq", [D, HH], BF16, kind="ExternalInput").ap()
    wk = nc.dram_tensor("wk", [D, HH], BF16, kind="ExternalInput").ap()
    wv = nc.dram_tensor("wv", [D, HH], BF16, kind="ExternalInput").ap()
    wo = nc.dram_tensor("wo", [HH, D], BF16, kind="ExternalInput").ap()
    bq = nc.dram_tensor("bq", [HH], F32, kind="ExternalInput").ap()
    bk = nc.dram_tensor("bk", [HH], F32, kind="ExternalInput").ap()
    bv = nc.dram_tensor("bv", [HH], F32, kind="ExternalInput").ap()
    out = nc.dram_tensor("out", [S, D], F32, kind="ExternalOutput").ap()

    NT = S // 512        # 512-token chunks
    NSK = S // 128       # 128-token key tiles
    NKT = D // 128       # 128-wide d_model tiles
    NKB = HH // 128      # 128-wide hidden tiles (also head pairs)
    NG = NSK // 2        # 2-key-tile score groups

    with tile.TileContext(nc) as tc:
        from contextlib import ExitStack

        for rep in range(reps):
          with ExitStack() as ctx:
            persist = ctx.enter_context(tc.tile_pool(name="persist", bufs=1))
            wpool = ctx.enter_context(tc.tile_pool(name="wpool", bufs=3))
            xpool = ctx.enter_context(tc.tile_pool(name="xpool", bufs=3))
            ptpool = ctx.enter_context(tc.tile_pool(name="ptpool", bufs=8))
            rpool = ctx.enter_context(tc.tile_pool(name="rpool", bufs=2))
            opool = ctx.enter_context(tc.tile_pool(name="opool", bufs=2))

            qt_sb = persist.tile([128, NKB, S], BF16, tag="qt")
            kt_sb = persist.tile([128, NKB, S], BF16, tag="kt")
            vb_sb = persist.tile([128, NSK, NHL * (HD + 1)], BF16, tag="vb")
            zt_sb = persist.tile([128, NKB, S], BF16, tag="zt")
            wo_sb = persist.tile([128, NKB, D], BF16, tag="wo")
            bq_sb = persist.tile([128, NKB], F32, tag="bq")
            bk_sb = persist.tile([128, NKB], F32, tag="bk")
            bvb_sb = persist.tile([128, HH], F32, tag="bvb")

            # bias/V-bias DMAs are emitted later, interleaved with the first
            # projection chunks, so the wk + first-xt DMAs (which gate the
            # first matmul) get the HBM bandwidth first.
            bv_bcast_in = bass.AP(tensor=bv.tensor, offset=bv.offset,
                                  ap=[[0, 128], [1, HH]])
            # ones columns of V~ (softmax denominator trick)
            ones_view = vb_sb.rearrange("p s (h dd) -> p s h dd", dd=HD + 1)[:, :, :, HD:HD + 1]
            nc.vector.memset(ones_view, 1.0)

            def load_w(w_dram, name):
                w_sb = wpool.tile([128, NKT, HH], BF16, tag="w", name=name)
                nc.sync.dma_start(
                    out=w_sb,
                    in_=w_dram.rearrange("(kt p) n -> p kt n", p=128))
                return w_sb

            def proj_chunk(xT, w_sb, dst, bias, t, pool, bufs_tag):
                """One 512-token chunk of a K/Q projection into dst[:, :, t]."""
                xt = xpool.tile([128, NKT, 512], BF16, tag="xt")
                nc.sync.dma_start(
                    out=xt,
                    in_=xT.rearrange("(kt p) s -> p kt s", p=128)[:, :, t * 512:(t + 1) * 512])
                for kb in range(NKB):
                    ps = pool.tile([128, 512], F32, tag=bufs_tag)
                    for kt in range(NKT):
                        nc.tensor.matmul(
                            ps,
                            lhsT=w_sb[:, kt, kb * 128:(kb + 1) * 128],
                            rhs=xt[:, kt, :],
                            start=(kt == 0), stop=(kt == NKT - 1))
                    nc.vector.tensor_scalar_add(
                        dst[:, kb, t * 512:(t + 1) * 512], ps,
                        bias[:, kb:kb + 1])

            def v_chunk(wv_sb, t, pool, bufs_tag):
                """One 512-token chunk of the V projection (natural layout,
                65-wide head groups) into vb_sb."""
                xt = xpool.tile([128, NKT, 512], BF16, tag="xt")
                nc.sync.dma_start(
                    out=xt,
                    in_=xvT.rearrange("(kt p) s -> p kt s", p=128)[:, :, t * 512:(t + 1) * 512])
                for m in range(4):
                    ps = pool.tile([128, 512], F32, tag=bufs_tag)
                    for kt in range(NKT):
                        nc.tensor.matmul(
                            ps,
                            lhsT=xt[:, kt, m * 128:(m + 1) * 128],
                            rhs=wv_sb[:, kt, :],
                            start=(kt == 0), stop=(kt == NKT - 1))
                    sk = t * 4 + m
                    vdst = vb_sb[:, sk, :].rearrange(
                        "p (h dd) -> p h dd", dd=HD + 1)[:, :, 0:HD]
                    nc.vector.tensor_add(
                        vdst,
                        ps.rearrange("p (h d) -> p h d", d=HD),
                        bvb_sb.rearrange("p (h d) -> p h d", d=HD))

            def attention(hp, c, spool, zpool):
                """Heads 2hp,2hp+1 x query chunk c: scores (paired), exp, zT."""
                zps = [zpool.tile([HD + 1, 512], F32, tag=f"z{d}",
                                  name=f"zps{d}_{hp}_{c}_{rep}", bufs=1)
                       for d in range(2)]
                for g in range(NG):
                    sps = [spool.tile([128, 2, 512], F32, tag=f"s{d}", bufs=1,
                                      name=f"sp{d}")
                           for d in range(2)]
                    pts = [ptpool.tile([128, 2, 512], BF16, tag=f"pt{d}",
                                       name=f"pt{d}")
                           for d in range(2)]
                    for j in range(2):
                        sk = 2 * g + j
                        for d in range(2):
                            nc.tensor.matmul(
                                sps[d][:, j, :],
                                lhsT=kt_sb[d * 64:(d + 1) * 64, hp,
                                           sk * 128:(sk + 1) * 128],
                                rhs=qt_sb[d * 64:(d + 1) * 64, hp,
                                          c * 512:(c + 1) * 512],
                                start=True, stop=True)
                    for d in range(2):
                        nc.scalar.activation(pts[d], sps[d], ACT.Exp,
                                             scale=0.125)
                    for d in range(2):
                        h = 2 * hp + d
                        for j in range(2):
                            sk = 2 * g + j
                            nc.tensor.matmul(
                                zps[d],
                                lhsT=vb_sb[:, sk, h * (HD + 1):(h + 1) * (HD + 1)],
                                rhs=pts[d][:, j, :],
                                start=(sk == 0), stop=(sk == NSK - 1))
                for d in range(2):
                    # evict z~ to SBUF so the PSUM bank frees; normalize
                    # from SBUF.  custom-DVE recip can't read base_partition
                    # 64: stage the denominator row at partition 0 first.
                    zr = rpool.tile([HD + 1, 512], F32, tag="zr")
                    nc.vector.tensor_copy(zr, zps[d])
                    dn = rpool.tile([1, 512], F32, tag="dn")
                    nc.vector.tensor_copy(dn, zr[HD:HD + 1, :])
                    rc = rpool.tile([1, 512], F32, tag="rc")
                    nc.vector.reciprocal_approx_fast(rc, dn)
                    bc = rpool.tile([HD, 512], F32, tag="bc")
                    nc.gpsimd.partition_broadcast(bc, rc, channels=HD)
                    nc.vector.tensor_mul(
                        zt_sb[d * 64:d * 64 + HD, hp, c * 512:(c + 1) * 512],
                        zr[0:HD, :], bc)

            def outproj(c, pools_tags):
                """Out-projection for query chunk c's 4 token tiles;
                pools_tags is a list of (pool, tag) PSUM slots to rotate
                through (2 slots -> matmul/evict pipelining)."""
                for t4 in range(4):
                    t = c * 4 + t4
                    os_t = opool.tile([128, D], F32, tag="os")
                    for n in range(D // 512):
                        pool, bufs_tag = pools_tags[(2 * t4 + n) % len(pools_tags)]
                        po = pool.tile([128, 512], F32, tag=bufs_tag, bufs=1,
                                       name="po")
                        for hb in range(NKB):
                            nc.tensor.matmul(
                                po,
                                lhsT=zt_sb[:, hb, t * 128:(t + 1) * 128],
                                rhs=wo_sb[:, hb, n * 512:(n + 1) * 512],
                                start=(hb == 0), stop=(hb == NKB - 1))
                        nc.vector.tensor_copy(os_t[:, n * 512:(n + 1) * 512], po)
                    nc.sync.dma_start(out=out[t * 128:(t + 1) * 128, :], in_=os_t)

            # ---- emission plan (priority = emission order):
            # score/z PSUM pools open up-front (6 banks) so attention(c0) can
            # start the moment KT(t0)+QT(c0) land; the 2-bank p1a covers the
            # lead-in projections, then hands its banks to pp/pout.  V is
            # interleaved with the remaining KT chunks so the c0 z matmuls
            # unblock early and recycle the bf16 probs tiles the exp stream
            # needs.  attention(c0) is emitted at priority 0 so the PE
            # prefers score matmuls over projection filler the moment their
            # operands are ready.
            with ExitStack() as c2:
                spool = c2.enter_context(
                    tc.tile_pool(name="spool", bufs=1, space="PSUM"))
                zpool = c2.enter_context(
                    tc.tile_pool(name="zpool", bufs=1, space="PSUM"))

                nc.sync.dma_start(out=bk_sb, in_=bk.rearrange("(kb p) -> p kb", p=128))
                wk_sb = load_w(wk, "w_k")
                with tc.tile_pool(name="p1a", bufs=2, space="PSUM") as p1a:
                    proj_chunk(xkT, wk_sb, kt_sb, bk_sb, 0, p1a, "ps1")
                    nc.sync.dma_start(out=bq_sb, in_=bq.rearrange("(kb p) -> p kb", p=128))
                    wq_sb = load_w(wq, "w_q")
                    proj_chunk(xqT, wq_sb, qt_sb, bq_sb, 0, p1a, "ps1")
                    nc.sync.dma_start(out=bvb_sb, in_=bv_bcast_in)
                    wv_sb = load_w(wv, "w_v")
                    proj_chunk(xkT, wk_sb, kt_sb, bk_sb, 1, p1a, "ps1")
                    v_chunk(wv_sb, 0, p1a, "ps1")
                    proj_chunk(xkT, wk_sb, kt_sb, bk_sb, 2, p1a, "ps1")
                    v_chunk(wv_sb, 1, p1a, "ps1")
                    proj_chunk(xkT, wk_sb, kt_sb, bk_sb, 3, p1a, "ps1")
                    v_chunk(wv_sb, 2, p1a, "ps1")
                    v_chunk(wv_sb, 3, p1a, "ps1")

                pp = c2.enter_context(
                    tc.tile_pool(name="pp", bufs=1, space="PSUM"))
                pout = c2.enter_context(
                    tc.tile_pool(name="pout", bufs=1, space="PSUM"))
                nc.sync.dma_start(
                    out=wo_sb,
                    in_=wo.rearrange("(hb p) n -> p hb n", p=128))

                with tc.high_priority():
                    for hp in range(NKB):
                        attention(hp, 0, spool, zpool)

                for c in range(1, NT):
                    proj_chunk(xqT, wq_sb, qt_sb, bq_sb, c, pp, "p1b")
                    for hp in range(NKB):
                        attention(hp, c, spool, zpool)
                    outproj(c - 1, [(pout, "po")])
                outproj(NT - 1, [(pout, "po"), (pp, "p1b")])

    nc.compile()
    return nc


_NC_CACHE = {}


def _get_nc(S=S_FULL):
    if S not in _NC_CACHE:
        _NC_CACHE[S] = build_nc(S)
    return _NC_CACHE[S]


def make_in_maps(query, key, value, Wq, bq, Wk, bk, Wv, bv, Wo, bo):
    """Shard full inputs into 8 per-core input dicts."""
    f32 = lambda a: np.ascontiguousarray(np.asarray(a, dtype=np.float32))
    bf16 = lambda a: np.ascontiguousarray(
        np.asarray(jnp.asarray(a, dtype=jnp.bfloat16)))
    in_maps = []
    for core in range(N_CORES):
        b, hg = core // 2, core % 2
        sl = slice(hg * HH, (hg + 1) * HH)
        in_maps.append({
            "xqT": bf16(np.asarray(query)[b].T),
            "xkT": bf16(np.asarray(key)[b].T),
            "xvT": bf16(np.asarray(value)[b].T),
            "wq": bf16(np.asarray(Wq)[:, sl]),
            "wk": bf16(np.asarray(Wk)[:, sl]),
            "wv": bf16(np.asarray(Wv)[:, sl]),
            "wo": bf16(np.asarray(Wo)[sl, :]),
            "bq": f32(np.asarray(bq)[sl]),
            "bk": f32(np.asarray(bk)[sl]),
            "bv": f32(np.asarray(bv)[sl]),
        })
    return in_maps


def kernel(query, key, value, Wq, bq, Wk, bk, Wv, bv, Wo, bo, **run_kwargs):
    nc = _get_nc(S_FULL)
    in_maps = make_in_maps(query, key, value, Wq, bq, Wk, bk, Wv, bv, Wo, bo)
    res = run_bass_kernel_spmd(nc, in_maps, core_ids=list(range(N_CORES)),
                               **run_kwargs)
    bo_np = np.asarray(bo, dtype=np.float32)
    outs = [np.asarray(r["out"], dtype=np.float32) for r in res.results]
    full = np.stack([outs[2 * b] + outs[2 * b + 1] + bo_np for b in range(B)])
    return full.astype(np.float32)
